# revision 1
# baseline (speedup 1.0000x reference)
"""CTRNN policy kernel for Trainium2 (8 NeuronCores, batch-parallel).

Reference computation (per batch element b, B=64, N=1024, OBS=64, A=16):
    I = E[b] @ obs[b]
    repeat 10x:  y = tanh(gain*(v+bias))*mask
                 v = (v + DT/tau * (-v + W[b]@y + I)) * mask
    action[b] = D[b] @ v

Sharding: batch 64 -> 8 cores x 8 individuals, fully data parallel.

Per-core algorithm (algebraic refactor to minimize per-iteration work):
    am = DT/tau*mask, cm = (1-DT/tau)*mask
    Wf = diag(am) @ W @ diag(mask)   (host-folded)
    Ef = diag(am) @ E                (host-folded)
    bc = bias*(1-cm)                 (host)
    state vs = v + bias; then per iteration:
        y   = tanh(g * vs)
        vs' = cm*vs + Wf@y + (Ef@obs + bc)
    finally action = D @ (vs - bias)

Layout: vector index n = p*8 + c maps to SBUF [p=partition(128), c=free(8)].
The matvec Wf@y runs on TensorE as 16 float32r matmuls per iteration:
stationary = y column chunk [128,1], moving = transposed-W slab [128,512],
accumulating into PSUM [1,1024] (row layout, n-ordered), which is then
fused-added with (Ef@obs+bc) on VectorE and DMA-scattered back to [128,8].
"""

import os
import sys
from contextlib import ExitStack

import numpy as np

for _p in ("/opt/trn_rl_repo", "/root/.axon_site/_ro/trn_rl_repo"):
    if os.path.isdir(_p) and _p not in sys.path:
        sys.path.append(_p)

import concourse.bass as bass  # noqa: E402
import concourse.tile as tile  # noqa: E402
from concourse import bacc, mybir  # noqa: E402
from concourse.bass_utils import run_bass_kernel_spmd  # noqa: E402

DT = 0.1
ITERS = int(1.0 // DT)  # == 9: reference.py uses `int(1.0 // DT)`, and 1.0//0.1 == 9.0
B_FULL, N, OBS, ADIM = 64, 1024, 64, 16
NCORES = 8
BPC = B_FULL // NCORES  # individuals per core
P, CN = 128, 8          # n = p*8 + c
F32 = mybir.dt.float32
F32R = mybir.dt.float32r
GROUPS = [(0, 1, 2), (3, 4, 5), (6, 7)]


def make_pools(ctx, tc):
    return dict(
        const=ctx.enter_context(tc.tile_pool(name="const", bufs=1)),
        wpool=ctx.enter_context(tc.tile_pool(name="w", bufs=4)),
        etpool=ctx.enter_context(tc.tile_pool(name="et", bufs=2)),
        rowpool=ctx.enter_context(tc.tile_pool(name="row", bufs=3)),
        scat=ctx.enter_context(tc.tile_pool(name="scat", bufs=6)),
        tmp=ctx.enter_context(tc.tile_pool(name="tmp", bufs=6)),
        pspool=ctx.enter_context(tc.tile_pool(name="ps", bufs=3, space="PSUM")),
        psact=ctx.enter_context(tc.tile_pool(name="psa", bufs=1, space="PSUM")),
    )


def kernel_body(ctx, tc, ins, out_ap, iters=ITERS, pools=None, probe_no_chain=False):
    nc = tc.nc
    Tanh = mybir.ActivationFunctionType.Tanh
    add = mybir.AluOpType.add
    mult = mybir.AluOpType.mult
    sub = mybir.AluOpType.subtract

    p = pools if pools is not None else make_pools(ctx, tc)
    const, wpool, etpool, rowpool = p["const"], p["wpool"], p["etpool"], p["rowpool"]
    scat, tmp, pspool, psact = p["scat"], p["tmp"], p["pspool"], p["psact"]

    # ---- constants / persistent state ----
    obs_sb = const.tile([OBS, BPC], F32, tag="obs", name="obs")
    nc.sync.dma_start(obs_sb[:], ins["obsT"][:])
    # per-individual [1, N] row tiles at partition 0 (engine ops need aligned
    # start partitions); seeded with bc, then += Ef@obs on device
    ifb_sb = {}
    for b in range(BPC):
        ifb_sb[b] = const.tile([1, N], F32, tag=f"ifb{b}", name=f"ifb{b}")
        nc.sync.dma_start(ifb_sb[b][:], ins["bc"][b])
    dtr_sb = const.tile([P, BPC * CN * ADIM], F32, tag="dtr", name="dtr")  # [128, 1024]
    for b in range(BPC):
        nc.sync.dma_start(dtr_sb[:, b * CN * ADIM:(b + 1) * CN * ADIM], ins["DTr"][b])

    cm_sb, g_sb, bias_sb, vs_sb, y_sb = {}, {}, {}, {}, {}
    for b in range(BPC):
        cm_sb[b] = const.tile([P, CN], F32, tag=f"cm{b}", name=f"cm{b}")
        nc.sync.dma_start(cm_sb[b][:], ins["cm"][b])
        g_sb[b] = const.tile([P, CN], F32, tag=f"g{b}", name=f"g{b}")
        nc.sync.dma_start(g_sb[b][:], ins["g"][b])
        bias_sb[b] = const.tile([P, CN], F32, tag=f"bias{b}", name=f"bias{b}")
        nc.sync.dma_start(bias_sb[b][:], ins["biasS"][b])
        vs_sb[b] = const.tile([P, CN], F32, tag=f"vs{b}", name=f"vs{b}")
        nc.sync.dma_start(vs_sb[b][:], ins["vs0"][b])
        y_sb[b] = const.tile([P, CN], F32R, tag=f"y{b}", name=f"y{b}")

    act_sb = const.tile([1, BPC * ADIM], F32, tag="act", name="act")

    # ---- W loads (slot-limited by pool bufs; scheduler orders them) ----
    w_sb = {}
    for b in range(BPC):
        w_sb[b] = wpool.tile([P, CN * N], F32R, tag="w", name="w")
        nc.sync.dma_start(w_sb[b][:], ins["Wf"][b])

    # ---- per-individual setup: input current + initial y ----
    for b in range(BPC):
        et = etpool.tile([OBS, N], F32, tag="et", name="et")
        nc.sync.dma_start(et[:], ins["ET"][b])
        ip = pspool.tile([1, N], F32, tag="ps", name="ps")
        for h in range(2):
            nc.tensor.matmul(
                ip[0:1, h * 512:(h + 1) * 512],
                obs_sb[:, b:b + 1],
                et[:, h * 512:(h + 1) * 512],
                start=True, stop=True,
            )
        # Ifb[b] = (Ef@obs) + bc[b]   (in-place: tile was seeded with bc)
        nc.vector.tensor_tensor(ifb_sb[b][:], ip[0:1, :], ifb_sb[b][:], op=add)
        # y0 = tanh(g * vs0)
        t2 = tmp.tile([P, CN], F32, tag="t2", name="t2")
        nc.vector.tensor_tensor(t2[:], g_sb[b][:], vs_sb[b][:], op=mult)
        nc.scalar.activation(y_sb[b][:], t2[:], Tanh)

    # ---- recurrent loop: groups of individuals interleaved per iteration ----
    for group in GROUPS:
        for t in range(iters):
            for b in group:
                wy = pspool.tile([1, N], F32, tag="ps", name="ps")
                for c in range(CN):
                    yc = y_sb[b][:, c:c + 1]
                    for h in range(2):
                        nc.tensor.matmul(
                            wy[0:1, h * 512:(h + 1) * 512],
                            yc,
                            w_sb[b][:, c * N + h * 512: c * N + h * 512 + 512],
                            start=(c == 0), stop=(c == CN - 1),
                        )
                if probe_no_chain:
                    continue
                u_row = rowpool.tile([1, N], F32, tag="urow", name="urow")
                nc.vector.tensor_tensor(u_row[:], wy[0:1, :], ifb_sb[b][:], op=add)
                u = scat.tile([P, CN], F32, tag="u", name="u")
                nc.sync.dma_start(u[:], u_row[:])  # [1,1024] -> [128,8], n = p*8+c
                t1 = tmp.tile([P, CN], F32, tag="t1", name="t1")
                nc.vector.tensor_tensor(t1[:], cm_sb[b][:], vs_sb[b][:], op=mult)
                nc.vector.tensor_tensor(vs_sb[b][:], t1[:], u[:], op=add)
                if t < iters - 1:
                    t2 = tmp.tile([P, CN], F32, tag="t2", name="t2")
                    nc.vector.tensor_tensor(t2[:], g_sb[b][:], vs_sb[b][:], op=mult)
                    nc.scalar.activation(y_sb[b][:], t2[:], Tanh)

    # ---- decode: action = D @ (vs - bias) ----
    for b in range(BPC):
        vf = tmp.tile([P, CN], F32, tag="vf", name="vf")
        nc.vector.tensor_tensor(vf[:], vs_sb[b][:], bias_sb[b][:], op=sub)
        ap = psact.tile([1, ADIM], F32, tag="psa", name="psa")
        for c in range(CN):
            nc.tensor.matmul(
                ap[0:1, :],
                vf[:, c:c + 1],
                dtr_sb[:, b * CN * ADIM + c * ADIM: b * CN * ADIM + (c + 1) * ADIM],
                start=(c == 0), stop=(c == CN - 1),
            )
        nc.vector.tensor_copy(act_sb[0:1, b * ADIM:(b + 1) * ADIM], ap[0:1, :])
    nc.sync.dma_start(out_ap[:], act_sb[0:1, :])


def build_nc(iters=ITERS, reps=1, probe_no_chain=False):
    nc = bacc.Bacc(
        "TRN2", target_bir_lowering=False, debug=False, enable_asserts=False,
    )
    ins = {}
    for name, shape in [
        ("ET", [BPC, OBS, N]),
        ("DTr", [BPC, P, CN * ADIM]),
        ("obsT", [OBS, BPC]),
        ("vs0", [BPC, P, CN]),
        ("cm", [BPC, P, CN]),
        ("g", [BPC, P, CN]),
        ("biasS", [BPC, P, CN]),
        ("bc", [BPC, N]),
    ]:
        ins[name] = nc.dram_tensor(name, shape, F32, kind="ExternalInput").ap()
    ins["Wf"] = nc.dram_tensor("Wf", [BPC, P, CN * N], F32R, kind="ExternalInput").ap()
    out_ap = nc.dram_tensor("act", [BPC, ADIM], F32, kind="ExternalOutput").ap()

    with tile.TileContext(nc) as tc:
        with ExitStack() as ctx:
            pools = make_pools(ctx, tc)
            for _rep in range(reps):
                kernel_body(ctx, tc, ins, out_ap, iters, pools, probe_no_chain)
    nc.compile()
    return nc


def _round_tf32(x):
    """Round fp32 array to tf32 (10-bit mantissa), round-to-nearest-even."""
    u = x.view(np.uint32)
    u = u + (0x0FFF + ((u >> 13) & 1))
    u &= np.uint32(0xFFFFE000)
    return u.view(np.float32)


def prep_in_maps(obs, v0, tau, gain, bias, W, mask, E, D):
    f = np.float32
    obs, v0, tau, gain, bias, W, mask, E, D = [
        np.asarray(x, dtype=f) for x in (obs, v0, tau, gain, bias, W, mask, E, D)
    ]
    am = (DT / tau) * mask                    # [64, N]
    cm = (1.0 - DT / tau) * mask
    Wf = W * am[:, :, None] * mask[:, None, :]
    WT = np.ascontiguousarray(Wf.transpose(0, 2, 1)).reshape(B_FULL, P, CN * N)
    WT = _round_tf32(WT)
    ETp = np.ascontiguousarray((E * am[:, :, None]).transpose(0, 2, 1))  # [64, OBS, N]
    DTp = np.ascontiguousarray(D.transpose(0, 2, 1)).reshape(B_FULL, P, CN * ADIM)
    obsT = np.ascontiguousarray(obs.T)        # [OBS, 64]
    vs0 = np.ascontiguousarray((v0 + bias).reshape(B_FULL, P, CN))
    cmS = np.ascontiguousarray(cm.reshape(B_FULL, P, CN))
    gS = np.ascontiguousarray(gain.reshape(B_FULL, P, CN))
    bS = np.ascontiguousarray(bias.reshape(B_FULL, P, CN))
    bc = np.ascontiguousarray(bias * (1.0 - cm))  # [64, N]

    in_maps = []
    for core in range(NCORES):
        s = slice(core * BPC, (core + 1) * BPC)
        in_maps.append({
            "Wf": np.ascontiguousarray(WT[s]),
            "ET": np.ascontiguousarray(ETp[s]),
            "DTr": np.ascontiguousarray(DTp[s]),
            "obsT": np.ascontiguousarray(obsT[:, s]),
            "vs0": vs0[s], "cm": cmS[s], "g": gS[s], "biasS": bS[s],
            "bc": bc[s],
        })
    return in_maps


_NC_CACHE = None


def _get_nc():
    global _NC_CACHE
    if _NC_CACHE is None:
        _NC_CACHE = build_nc()
    return _NC_CACHE


def kernel(obs, v0, tau, gain, bias, W, mask, E, D):
    nc = _get_nc()
    in_maps = prep_in_maps(obs, v0, tau, gain, bias, W, mask, E, D)
    res = run_bass_kernel_spmd(nc, in_maps, core_ids=list(range(NCORES)))
    return np.concatenate([res.results[c]["act"] for c in range(NCORES)], axis=0)



# revision 8
# speedup vs baseline: 1.0128x; 1.0128x over previous
"""CTRNN policy kernel for Trainium2 (8 NeuronCores, batch-parallel).

Reference computation (per batch element b, B=64, N=1024, OBS=64, A=16):
    I = E[b] @ obs[b]
    repeat ITERS x:  y = tanh(gain*(v+bias))*mask
                     v = (v + DT/tau * (-v + W[b]@y + I)) * mask
    action[b] = D[b] @ v

Sharding: batch 64 -> 8 cores x 8 individuals, fully data parallel.

Algebraic refactor (all folds on host):
    am = DT/tau*mask, cm = (1-DT/tau)*mask
    s  = g*(v+bias)                  (state; g = gain, zero-guarded)
    Wg = diag(g*am) W diag(mask)     -> bf16 on device (SBUF-resident)
    Ig = g*(am*(E@obs) + bias*(1-cm))
    per iteration: y = tanh(s);  s' = cm*s + Wg@y + Ig
    action = (D/g) @ s_final - D@bias

Per-core schedule: 2 rounds x 4 individuals. The matvec for the 4
individuals of a round runs on the 4 PE column strips (tile_position
col-tiling): stationary = y column [128,1] bf16, moving = Wg^T slab
[128,512] bf16, outputs land as rows [1,512] at PSUM partitions
{0,32,64,96} of 2 shared banks. VectorE reads each full PSUM bank
[128,512] (fusing the +Ig add; non-row partitions are dead lanes at no
extra cost), PE transposes [128,128] blocks back to column layout, and
the per-individual state update [128,8] reads the transposed columns
at free-stride 128. No DMA in the recurrent loop.

Column layout per individual: n = p + 128*t stored at tile[p, t].
"""

import os
import sys
from contextlib import ExitStack

import numpy as np

for _p in ("/opt/trn_rl_repo", "/root/.axon_site/_ro/trn_rl_repo"):
    if os.path.isdir(_p) and _p not in sys.path:
        sys.path.append(_p)

import concourse.bass as bass  # noqa: E402
import concourse.tile as tile  # noqa: E402
from concourse import bacc, mybir  # noqa: E402
from concourse.bass_utils import run_bass_kernel_spmd  # noqa: E402

DT = 0.1
ITERS = int(1.0 // DT)  # == 9: reference.py uses `int(1.0 // DT)`, and 1.0//0.1 == 9.0
B_FULL, N, OBS, ADIM = 64, 1024, 64, 16
NCORES = 8
BPC = B_FULL // NCORES  # individuals per core
P = 128
NCH = 8                 # 128-chunks per vector
RQ = 4                  # individuals per round (one per PE column strip)
NR = 2                  # rounds
F32 = mybir.dt.float32
F32R = mybir.dt.float32r
BF16 = mybir.dt.bfloat16


def make_pools(ctx, tc):
    return dict(
        const=ctx.enter_context(tc.tile_pool(name="const", bufs=1)),
        wpool=ctx.enter_context(tc.tile_pool(name="w", bufs=BPC)),
        state=ctx.enter_context(tc.tile_pool(name="state", bufs=2)),
        prow=ctx.enter_context(tc.tile_pool(name="prow", bufs=4, space="PSUM")),
        ptr=ctx.enter_context(tc.tile_pool(name="ptr", bufs=2, space="PSUM")),
    )


def kernel_body(ctx, tc, ins, out_ap, iters=ITERS, pools=None):
    nc = tc.nc
    Tanh = mybir.ActivationFunctionType.Tanh
    add = mybir.AluOpType.add
    mult = mybir.AluOpType.mult
    sub = mybir.AluOpType.subtract

    p = pools if pools is not None else make_pools(ctx, tc)
    const, wpool, state = p["const"], p["wpool"], p["state"]
    prow, ptr = p["prow"], p["ptr"]

    # ---- constants ----
    ident_sb = const.tile([P, P], F32, tag="ident", name="ident")
    nc.sync.dma_start(ident_sb[:], ins["ident"][:])
    cm_sb = const.tile([P, BPC * NCH], F32, tag="cm", name="cm")
    nc.sync.dma_start(cm_sb[:], ins["cmc"][:])
    igp_sb = {}
    for r in range(NR):
        for j in range(2):
            igp_sb[r, j] = const.tile([P, 512], F32, tag=f"ig{r}{j}", name=f"ig{r}{j}")
            nc.sync.dma_start(igp_sb[r, j][:], ins["igp"][r][j])

    # ---- initial state + y0 (per-individual column tiles [128, 8]) ----
    s_cur, y_cur = [None] * BPC, [None] * BPC
    for b in range(BPC):
        s_t = state.tile([P, NCH], F32, tag=f"s{b}", name=f"s{b}")
        nc.sync.dma_start(s_t[:], ins["s0c"][:, NCH * b:NCH * b + NCH])
        y_t = state.tile([P, NCH], BF16, tag=f"y{b}", name=f"y{b}")
        nc.scalar.activation(y_t[:], s_t[:], Tanh)
        s_cur[b], y_cur[b] = s_t, y_t

    # ---- W loads (resident for the whole loop) ----
    w_sb = {}
    for b in range(BPC):
        w_sb[b] = wpool.tile([P, NCH * N], BF16, tag="w", name=f"w{b}")
        nc.sync.dma_start(w_sb[b][:], ins["Wsb"][b])

    # ---- decode constants (needed late; after W in DMA queue) ----
    dgt_sb = const.tile([P, BPC * P], F32, tag="dgt", name="dgt")
    nc.sync.dma_start(dgt_sb[:], ins["dgtc"][:])
    db0_sb = const.tile([ADIM, BPC], F32, tag="db0", name="db0")
    nc.sync.dma_start(db0_sb[:], ins["db0"][:])
    act_sb = const.tile([ADIM, BPC], F32, tag="act", name="act")

    # ---- recurrent loop ----
    for t in range(iters):
        # cm*s has no dependency on this iteration's matmuls: run first on DVE
        tmp = [None] * BPC
        for b in range(BPC):
            tm = state.tile([P, NCH], F32, tag=f"t{b}", name=f"t{b}")
            nc.vector.tensor_tensor(
                tm[:], cm_sb[:, NCH * b:NCH * b + NCH], s_cur[b][:], op=mult)
            tmp[b] = tm
        # matvec rows: 4 individuals per round on the 4 PE column strips
        prows = [None] * NR
        for r in range(NR):
            pr = [prow.tile([P, 512], F32, tag="pr", name="pr") for _ in range(2)]
            prows[r] = pr
            for h in range(NCH):
                for j in range(2):
                    for q in range(RQ):
                        b = RQ * r + q
                        nc.tensor.matmul(
                            pr[j][32 * q:32 * q + 1, :],
                            y_cur[b][:, h:h + 1],
                            w_sb[b][:, h * N + 512 * j: h * N + 512 * j + 512],
                            start=(h == 0), stop=(h == NCH - 1),
                            tile_position=(0, 32 * q),
                        )
        # full-bank PSUM read fused with +Ig -> SBUF [128, 512] (rows at 32q)
        us = [None] * NR
        for r in range(NR):
            u = [None, None]
            for j in range(2):
                u[j] = state.tile([P, 512], F32, tag=f"u{r}{j}", name=f"u{r}{j}")
                nc.vector.tensor_tensor(
                    u[j][:], prows[r][j][:], igp_sb[r, j][:], op=add)
            us[r] = u
        # PE transposes: [128,128] blocks; chunk t of individual q lands at
        # column 32q + 128*(t%4) of pt[j=t//4]
        pts = [None] * NR
        for r in range(NR):
            pt = ptr.tile([P, 2 * 512], F32, tag="pt", name="pt")
            for j in range(2):
                for t4 in range(4):
                    nc.tensor.transpose(
                        pt[:, 512 * j + 128 * t4: 512 * j + 128 * t4 + 128],
                        us[r][j][:, 128 * t4:128 * t4 + 128],
                        ident_sb[:],
                    )
            pts[r] = pt
        # per-individual state update + tanh
        for r in range(NR):
            for q in range(RQ):
                b = RQ * r + q
                s_n = state.tile([P, NCH], F32, tag=f"s{b}", name=f"s{b}")
                for j in range(2):
                    nc.vector.tensor_tensor(
                        s_n[:, 4 * j:4 * j + 4],
                        tmp[b][:, 4 * j:4 * j + 4],
                        pts[r][:, 512 * j + 32 * q: 512 * j + 32 * q + 385:128],
                        op=add,
                    )
                s_cur[b] = s_n
                if t < iters - 1:
                    y_n = state.tile([P, NCH], BF16, tag=f"y{b}", name=f"y{b}")
                    nc.scalar.activation(y_n[:], s_n[:], Tanh)
                    y_cur[b] = y_n

    # ---- decode: action = Dg @ s_final - D@bias ----
    # borrow a transpose-pool PSUM tile; individual b accumulates in column b
    pd = ptr.tile([P, 2 * 512], F32, tag="pt", name="pt")
    for b in range(BPC):
        for h in range(NCH):
            nc.tensor.matmul(
                pd[0:ADIM, b:b + 1],
                dgt_sb[:, P * b + ADIM * h: P * b + ADIM * h + ADIM],
                s_cur[b][:, h:h + 1],
                start=(h == 0), stop=(h == NCH - 1),
            )
    nc.vector.tensor_tensor(act_sb[:], pd[0:ADIM, 0:BPC], db0_sb[:], op=sub)
    nc.sync.dma_start(out_ap[:], act_sb[:])


def build_nc(iters=ITERS):
    nc = bacc.Bacc(
        "TRN2", target_bir_lowering=False, debug=False, enable_asserts=False,
    )
    ins = {}
    for name, shape, dt in [
        ("Wsb", [BPC, P, NCH * N], BF16),
        ("s0c", [P, BPC * NCH], F32),
        ("cmc", [P, BPC * NCH], F32),
        ("igp", [NR, 2, P, 512], F32),
        ("dgtc", [P, BPC * P], F32),
        ("db0", [ADIM, BPC], F32),
        ("ident", [P, P], F32),
    ]:
        ins[name] = nc.dram_tensor(name, shape, dt, kind="ExternalInput").ap()
    out_ap = nc.dram_tensor("act", [ADIM, BPC], F32, kind="ExternalOutput").ap()

    with tile.TileContext(nc) as tc:
        with ExitStack() as ctx:
            pools = make_pools(ctx, tc)
            kernel_body(ctx, tc, ins, out_ap, iters, pools)
    nc.compile()
    return nc


def prep_in_maps(obs, v0, tau, gain, bias, W, mask, E, D):
    f = np.float32
    obs, v0, tau, gain, bias, W, mask, E, D = [
        np.asarray(x, dtype=f) for x in (obs, v0, tau, gain, bias, W, mask, E, D)
    ]
    import ml_dtypes
    bf16 = ml_dtypes.bfloat16

    g = np.where(gain == 0.0, f(1e-6), gain)    # exact-rescaling guard
    am = (DT / tau) * mask                      # [64, N]
    cm = (1.0 - DT / tau) * mask
    I = np.einsum("bno,bo->bn", E, obs)         # [64, N]
    Ig = g * (am * I + bias * (1.0 - cm))
    s0 = g * (v0 + bias)
    Wg = W * (g * am)[:, :, None] * mask[:, None, :]
    # device layout: w[b][k, h*N + n] = Wg[b, n, 128h+k]
    WgT = Wg.transpose(0, 2, 1)                 # [b, m, n]
    wdev = np.ascontiguousarray(
        WgT.reshape(B_FULL, NCH, P, N).transpose(0, 2, 1, 3)
    ).reshape(B_FULL, P, NCH * N).astype(bf16)

    def cols(x):  # [64, N] -> [core, p, 8*b_local + t]  (n = p + 128 t)
        xc = x.reshape(NCORES, BPC, NCH, P)
        return np.ascontiguousarray(xc.transpose(0, 3, 1, 2)).reshape(
            NCORES, P, BPC * NCH)

    s0c = cols(s0)
    cmc = cols(cm)
    # padded Ig rows: igp[core, r, j, 32q, :] = Ig[8core+4r+q, 512j:512j+512]
    igp = np.zeros((NCORES, NR, 2, P, 512), f)
    for r in range(NR):
        for j in range(2):
            for q in range(RQ):
                igp[:, r, j, 32 * q, :] = Ig.reshape(NCORES, BPC, N)[
                    :, RQ * r + q, 512 * j:512 * j + 512]
    Dg = D / g[:, None, :]
    dgt = np.ascontiguousarray(
        Dg.transpose(0, 2, 1).reshape(B_FULL, NCH, P, ADIM).transpose(0, 2, 1, 3)
    ).reshape(B_FULL, P, P)
    dgtc = np.ascontiguousarray(
        dgt.reshape(NCORES, BPC, P, P).transpose(0, 2, 1, 3)
    ).reshape(NCORES, P, BPC * P)
    db0 = np.einsum("ban,bn->ba", D, bias)
    db0c = np.ascontiguousarray(db0.reshape(NCORES, BPC, ADIM).transpose(0, 2, 1))
    ident = np.eye(P, dtype=f)

    in_maps = []
    for core in range(NCORES):
        s = slice(core * BPC, (core + 1) * BPC)
        in_maps.append({
            "Wsb": np.ascontiguousarray(wdev[s]),
            "s0c": s0c[core], "cmc": cmc[core], "igp": igp[core],
            "dgtc": dgtc[core], "db0": db0c[core], "ident": ident,
        })
    return in_maps


_NC_CACHE = None


def _get_nc():
    global _NC_CACHE
    if _NC_CACHE is None:
        _NC_CACHE = build_nc()
    return _NC_CACHE


def kernel(obs, v0, tau, gain, bias, W, mask, E, D):
    nc = _get_nc()
    in_maps = prep_in_maps(obs, v0, tau, gain, bias, W, mask, E, D)
    res = run_bass_kernel_spmd(nc, in_maps, core_ids=list(range(NCORES)))
    # device output is [ADIM, BPC] per core
    return np.concatenate(
        [np.ascontiguousarray(res.results[c]["act"].T) for c in range(NCORES)],
        axis=0,
    )
